# revision 32
# baseline (speedup 1.0000x reference)
"""MetaNetImageEncoder Trainium2 kernel — fp8 DoubleRow edition.

Data-parallel over batch: 8 samples per NeuronCore x 8 cores.

Per core (sample-local b in 0..7, D=768, N=196 patches, T=8 tasks):
  1. base pass:   A = P @ W1 as fp8 DoubleRow matmuls (K=256 per instr),
                  pooled_b = sum_n relu(A + b1') via ACT/DVE accum_out
                  (b1' = b1/(SX*SW); the fp8 scale folds into later muls)
  2. MetaNet:     coefs via small-stationary matmuls + PE transposes
  3. mixing:      M_b = sum_t c[t,b] dW1[t] with a (t,s32)-packed fp8
                  DoubleRow stationary; 4 samples x 32 i-rows per pass;
                  PSUM evacuated by DVE/GpSimd/ACT round-robin (x1/16)
  4. de-interleave: wide 32-partition DMAs regroup mixing output into
                  per-sample DR-layout stationary tiles
  5. final pass:  pf = P@W1 + P@M_b (6 fp8 DR matmuls, one PSUM chain),
                  relu-pool split between ACT and DVE
  6. layer 2:     out = pooled@W2 (bf16) + sum_t cdiag (pooled@dW2 fp8 DR)
                  + coefs@db2 + b2
"""
import numpy as np
import ml_dtypes

import concourse.bass as bass
import concourse.mybir as mybir
import concourse.tile as tile
from concourse.vector_clock import ScopedClock
from concourse.bass_utils import run_bass_kernel_spmd

F32 = mybir.dt.float32
BF16 = mybir.dt.bfloat16
F8 = mybir.dt.float8e4
RELU = mybir.ActivationFunctionType.Relu
DR = mybir.MatmulPerfMode.DoubleRow
ADD = mybir.AluOpType.add
MAX = mybir.AluOpType.max
MULT = mybir.AluOpType.mult

P = 16
D = 768
T = 8
HM = 192
NPAT = 196          # 14*14 patches
B = 64
NCORES = 8
BC = B // NCORES    # 8 samples per core
NB = BC * NPAT      # 1568
KT = D // 128       # 6 k-tiles
KTD = 3             # 3 double k-tiles

SX = 3.0            # patch fp8 scale
SW = 0.06           # weight fp8 scale
SXW = SX * SW

_PATCHED = False


def _apply_tile_patch():
    """This container's walrus allows only one sem wait per instruction;
    TileContext's exit drain attaches one wait per live semaphore. Split
    them onto standalone single-wait nops."""
    global _PATCHED
    if _PATCHED:
        return
    _PATCHED = True

    def _patched(self, tick_clock, wait_clock):
        carrier = self.nc.sync.nop(nofuse=True, hint="drain_waits")
        wait_clock.add_sem_waits(
            carrier.ins, ScopedClock({None: tick_clock.global_clock})
        )
        si = carrier.ins.sync_info
        waits = list(si.on_wait) if si else []
        if len(waits) > 1:
            carrier.ins.sync_info = mybir.SyncInfo(on_wait=[waits[0]], on_update=[])
            for w in waits[1:]:
                extra = self.nc.sync.nop(nofuse=True, hint="drain_waits")
                extra.ins.sync_info = mybir.SyncInfo(on_wait=[w], on_update=[])
        self.nc.sync.drain()
        self.nc.all_engine_barrier()
        popped = self.nc._tile_sem_poison_stack.pop()
        assert popped is self._sem_poison
        self.nc.clear_and_free_semaphores(list(self.sems.allocated().values()))
        self.nc.all_engine_barrier()

    tile.TileContext._drain_and_barrier = _patched


def _split_multi_waits(nc, max_waits: int = 1):
    """Hoist extra sem waits onto same-engine InstNoOp carriers."""
    for f in nc.m.functions:
        for blk in f.blocks:
            out = []
            for inst in blk.instructions:
                si = inst.sync_info
                if si is not None and len(si.on_wait) > max_waits:
                    waits = list(si.on_wait)
                    for i, w in enumerate(waits[:-max_waits]):
                        out.append(mybir.InstNoOp(
                            name=f"{inst.name}-w{i}",
                            sync_info=mybir.SyncInfo(on_wait=[w], on_update=[]),
                            bass_nofuse=True,
                            engine=inst.engine,
                        ))
                    inst.sync_info = mybir.SyncInfo(
                        on_wait=waits[-max_waits:], on_update=list(si.on_update)
                    )
                out.append(inst)
            blk.instructions = out


def build_kernel(split_waits=True, debug=False):
    nc = bass.Bass(target_bir_lowering=False, trn_type="TRN2")

    din = {}
    def inp(name, shape, dt):
        din[name] = nc.dram_tensor(name, shape, dt, kind="ExternalInput")
        return din[name]

    xt = inp("xt", (128, KTD, 2, NB), F8)        # patches^T/SX, DR layout
    w1 = inp("w1", (128, KTD, 2, D), F8)         # W1/SW, DR layout
    w2 = inp("w2", (128, KT, D), BF16)           # W2 [k_local, kt, e]
    dw1 = inp("dw1", (128, 24, 2, D), F8)        # [(t,slo), iblk, shi, j]
    w1i = inp("w1i", (128, 4, 2, D), F8)         # W1/SW mixing-layout, ktd0
    dw2 = inp("dw2", (128, T, KTD, 2, D), F8)    # [k_local, t, ktd, hi, e]
    db1 = inp("db1", (T, D), BF16)               # db1/(SX*SW)
    db2 = inp("db2", (T, D), BF16)
    b1t = inp("b1t", (128, KT), F32)             # b1/(SX*SW) [j_local, jt]
    b2t = inp("b2t", (128, KT), F32)             # b2 [e_local, et]
    b2t4 = inp("b2t4", (128, KT, 4), F32)        # b2 bcast over 4 samples
    b2r = inp("b2r", (BC, D), F32)               # b2 replicated over samples
    mw1 = inp("mw1", (128, KT, HM), BF16)
    mb1t = inp("mb1t", (128, 2), F32)            # mb1 [h_local, chunk]
    mw2 = inp("mw2", (128, 2, T), BF16)          # [h_local, g, t], g=1 padded
    mb2r = inp("mb2r", (BC, T), F32)             # mb2 replicated over samples
    iexp16 = inp("iexp16", (T, 128), BF16)       # 16*repeat(eye(8),16,axis=1)
    mask32 = inp("mask32", (128, 2, 32), BF16)   # [(t,slo), shi, s'32]
    i8 = inp("i8", (T, T), F32)                  # eye(8)
    i8bf = inp("i8bf", (T, T), BF16)             # eye(8) bf16

    out = nc.dram_tensor("out", (BC, D), F32, kind="ExternalOutput")
    if debug:
        for nm, shp, dt in [
                ("dbg_poolb", (128, KT, BC), F32), ("dbg_coefsB", (BC, T), F32),
                ("dbg_mxcb0", (128, 4, KTD, 2, D), F8),
                ("dbg_pooln", (128, KT, BC), F32),
                ("dbg_vst", (BC, T, D), BF16)]:
            din[nm] = nc.dram_tensor(nm, shp, dt, kind="ExternalOutput")

    with tile.TileContext(nc) as tc:
        with (
            tc.tile_pool(name="big", bufs=1) as big,
            tc.tile_pool(name="sm", bufs=1) as sm,
            tc.tile_pool(name="scr", bufs=2) as scr,
        ):
            # ---------- persistent loads ----------
            w1_sb = big.tile([128, KTD, 2, D], F8, tag="w1")
            nc.sync.dma_start(w1_sb[:], w1[:])
            xt_sb = big.tile([128, KTD, 2, NB], F8, tag="xt")
            nc.scalar.dma_start(xt_sb[:], xt[:])
            b1t_sb = sm.tile([128, KT], F32, tag="b1t")
            nc.sync.dma_start(b1t_sb[:], b1t[:])
            b2t_sb = sm.tile([128, KT], F32, tag="b2t")
            nc.sync.dma_start(b2t_sb[:], b2t[:])
            b2r_sb = sm.tile([BC, D], F32, tag="b2r")
            nc.sync.dma_start(b2r_sb[:], b2r[:])
            mw1_sb = sm.tile([128, KT, HM], BF16, tag="mw1")
            nc.sync.dma_start(mw1_sb[:], mw1[:])
            mb1t_sb = sm.tile([128, 2], F32, tag="mb1t")
            nc.sync.dma_start(mb1t_sb[:], mb1t[:])
            mw2_sb = sm.tile([128, 2, T], BF16, tag="mw2")
            nc.sync.dma_start(mw2_sb[:], mw2[:])
            mb2r_sb = sm.tile([BC, T], F32, tag="mb2r")
            nc.sync.dma_start(mb2r_sb[:], mb2r[:])
            b2t4_sb = sm.tile([128, KT, 4], F32, tag="b2t4")
            nc.sync.dma_start(b2t4_sb[:], b2t4[:])
            iexp16_sb = sm.tile([T, 128], BF16, tag="iexp16")
            nc.sync.dma_start(iexp16_sb[:], iexp16[:])
            mask32_sb = sm.tile([128, 2, 32], BF16, tag="mask32")
            nc.sync.dma_start(mask32_sb[:], mask32[:])
            i8_sb = sm.tile([T, T], F32, tag="i8")
            nc.sync.dma_start(i8_sb[:], i8[:])
            i8bf_sb = sm.tile([T, T], BF16, tag="i8bf")
            nc.sync.dma_start(i8bf_sb[:], i8bf[:])
            db1_sb = sm.tile([T, D], BF16, tag="db1")
            nc.sync.dma_start(db1_sb[:], db1[:])
            db2_sb = sm.tile([T, D], BF16, tag="db2")
            nc.sync.dma_start(db2_sb[:], db2[:])
            # scalar queue: w2 (metanet), then mixing/ph5 tensors
            w2_sb = big.tile([128, KT, D], BF16, tag="w2")
            nc.scalar.dma_start(w2_sb[:], w2[:])
            dw1_sb = big.tile([128, 24, 2, D], F8, tag="dw1")
            nc.scalar.dma_start(dw1_sb[:], dw1[:])
            w1i_sb = big.tile([128, 4, 2, D], F8, tag="w1i")
            nc.scalar.dma_start(w1i_sb[:], w1i[:])
            dw2_sb = big.tile([128, T, KTD, 2, D], F8, tag="dw2")
            nc.scalar.dma_start(dw2_sb[:], dw2[:])

            poolb = sm.tile([128, KT, BC], F32, tag="poolb")
            pooln = sm.tile([128, KT, BC], F32, tag="pooln")
            zeros_sb = sm.tile([128, NPAT], F8, tag="zeros")
            nc.vector.memset(zeros_sb[:], 0.0)
            zeros32_sb = sm.tile([128, 2, 32], F8, tag="zeros32")
            nc.vector.memset(zeros32_sb[:], 0.0)

            def relu_pool(pa, bi, jt, b, dst):
                """relu(pa_slice + b1') summed into dst column; ACT or DVE."""
                sl = pa[:, bi * NPAT:(bi + 1) * NPAT]
                if b % 2 == 0:
                    ro = scr.tile([128, NPAT], F8, tag="ro")
                    nc.scalar.activation(
                        ro[:], sl, RELU, bias=b1t_sb[:, jt:jt + 1],
                        accum_out=dst)
                else:
                    ro = scr.tile([128, NPAT], F8, tag="ro")
                    nc.vector.scalar_tensor_tensor(
                        ro[:], sl, b1t_sb[:, jt:jt + 1], zeros_sb[:],
                        op0=ADD, op1=MAX, accum_out=dst)

            # ---------- phase 1 + MetaNet, pipelined per 4-sample group ----
            poolb_bf = sm.tile([128, KT, BC], BF16, tag="poolbbf")
            cb2_0 = sm.tile([128, 2, 128], F8, tag="cb2_0")
            cb2_1 = sm.tile([128, 2, 128], F8, tag="cb2_1")
            cb2 = [cb2_0, cb2_1]
            coefsT_bf = sm.tile([T, T], BF16, tag="coefsTbf")
            coefsB = sm.tile([T, T], F32, tag="coefsB")
            with (tc.tile_pool(name="psA", bufs=4, space="PSUM") as psA,
                  tc.tile_pool(name="psB", bufs=1, space="PSUM") as psB):
                for g in range(2):
                    gs = slice(g * 4, g * 4 + 4)
                    for jt in range(KT):
                        for ch2 in range(2):   # 392-chunks of this group
                            ch = g * 2 + ch2
                            pa = psA.tile([128, 392], F32, tag="a")
                            for ktd in range(KTD):
                                nc.tensor.matmul(
                                    pa[:],
                                    w1_sb[:, ktd, :, jt * 128:(jt + 1) * 128],
                                    xt_sb[:, ktd, :, ch * 392:(ch + 1) * 392],
                                    start=(ktd == 0), stop=(ktd == KTD - 1),
                                    perf_mode=DR)
                            ro = scr.tile([128, 2, NPAT], BF16, tag="rr")
                            if not (ch2 == 1 and jt % 2 == 1):
                                nc.scalar.activation(
                                    ro[:],
                                    pa[:].rearrange("p (b n) -> p b n", b=2),
                                    RELU, bias=b1t_sb[:, jt:jt + 1])
                            else:
                                nc.vector.tensor_scalar(
                                    ro[:],
                                    pa[:].rearrange("p (b n) -> p b n", b=2),
                                    b1t_sb[:, jt:jt + 1], 0.0,
                                    op0=ADD, op1=MAX)
                            nc.vector.tensor_reduce(
                                poolb[:, jt, ch * 2:(ch + 1) * 2], ro[:],
                                axis=mybir.AxisListType.X, op=ADD)
                        nc.gpsimd.tensor_scalar_mul(
                            poolb_bf[:, jt, gs], poolb[:, jt, gs], SXW / NPAT)
                    base2b = sm.tile([4, D], BF16, tag="base2b")
                    for eh in range(2):
                        pb2 = psB.tile([4, 512], F32, tag="b2big")
                        for kt in range(KT):
                            nc.tensor.matmul(
                                pb2[:, 0:384], poolb_bf[:, kt, gs],
                                w2_sb[:, kt, eh * 384:(eh + 1) * 384],
                                start=(kt == 0), stop=(kt == KT - 1))
                        nc.vector.tensor_copy(
                            base2b[:, eh * 384:(eh + 1) * 384], pb2[:, 0:384])

                    # transpose to [e_local, et, b4] + b2 bias in one add
                    tpp = psB.tile([128, KT, 4], BF16, tag="tp")
                    for et in range(KT):
                        nc.tensor.transpose(
                            tpp[:, et, :], base2b[:, et * 128:(et + 1) * 128],
                            i8bf_sb[0:4, 0:4])
                    base2T = sm.tile([128, KT, 4], BF16, tag="base2T")
                    nc.vector.tensor_tensor(
                        base2T[:], tpp[:], b2t4_sb[:], op=ADD)

                    # mh^T[h, b4] directly
                    pmh = psB.tile([128, T], F32, tag="mh")
                    for et in range(KT):
                        nc.tensor.matmul(
                            pmh[:, 0:4], mw1_sb[:, et, 0:128],
                            base2T[:, et, :],
                            start=(et == 0), stop=(et == KT - 1))
                    for et in range(KT):
                        nc.tensor.matmul(
                            pmh[0:64, 4:8], mw1_sb[:, et, 128:192],
                            base2T[:, et, :],
                            start=(et == 0), stop=(et == KT - 1))
                    mhT0 = sm.tile([128, 4], BF16, tag="mhT0")
                    mhT1 = sm.tile([64, 4], BF16, tag="mhT1")
                    nc.vector.tensor_scalar(
                        mhT0[:], pmh[:, 0:4], mb1t_sb[:, 0:1], 0.0,
                        op0=ADD, op1=MAX)
                    nc.vector.tensor_scalar(
                        mhT1[:], pmh[0:64, 4:8], mb1t_sb[0:64, 1:2], 0.0,
                        op0=ADD, op1=MAX)

                    # coefs[b4, t]
                    pcB = psB.tile([4, T], F32, tag="sm8")
                    nc.tensor.matmul(pcB[:], mhT0[:], mw2_sb[:, 0, :],
                                     start=True, stop=False)
                    nc.tensor.matmul(pcB[:], mhT1[:], mw2_sb[0:64, 1, :],
                                     start=False, stop=True)
                    coefsBs = sm.tile([4, T], F32, tag="coefsBs")
                    nc.vector.tensor_tensor(
                        coefsBs[:], pcB[:], mb2r_sb[0:4, :], op=ADD)

                    # coefsT[t, b4] via PE transpose; bf16 column slice
                    ptc = psB.tile([T, 4], F32, tag="sm8")
                    nc.tensor.transpose(ptc[:], coefsBs[:], i8_sb[0:4, 0:4])
                    nc.vector.tensor_copy(coefsT_bf[:, gs], ptc[:])

                    # crep16[(t,slo), b4] then mixing stationary cb2_g
                    pcr = psB.tile([128, 4], F32, tag="sm8")
                    nc.tensor.matmul(pcr[:], iexp16_sb[:], coefsT_bf[:, gs],
                                     start=True, stop=True)
                    for bg in range(4):
                        nc.vector.scalar_tensor_tensor(
                            cb2[g][:, :, bg * 32:(bg + 1) * 32],
                            mask32_sb[:], pcr[:, bg:bg + 1],
                            zeros32_sb[:], op0=MULT, op1=ADD)

                # full coefsB[b, t] from coefsT_bf via one transpose
                pcd = psB.tile([T, T], BF16, tag="sm8")
                nc.tensor.transpose(pcd[:], coefsT_bf[:], i8bf_sb[:])
                nc.vector.tensor_copy(coefsB[:], pcd[:])

            # ---------- phase 3: mixing (fp8 DR, 4 samples x 32 rows) ----------
            # mxg[g][(bg,s'), ph, kh, j] = M[(g,bg), i=(kh*4+ph)*32+s', j]/SW
            mxgp = tc.alloc_tile_pool(name="mxgp", bufs=1)
            mxg_0 = mxgp.tile([128, 4, KT, D], F8, tag="mxg0")
            mxg_1 = mxgp.tile([128, 4, KT, D], F8, tag="mxg1")
            mxg = [mxg_0, mxg_1]
            mxcb_0 = big.tile([128, 4, KTD, 2, D], F8, tag="mxcb0")
            mxcb_1 = big.tile([128, 4, KTD, 2, D], F8, tag="mxcb1")
            mxcb = [mxcb_0, mxcb_1]
            nb1t = sm.tile([128, KT, BC], F32, tag="nb1t")
            cdiag = sm.tile([T, T, T], BF16, tag="cdiag")
            with (tc.tile_pool(name="psM", bufs=3, space="PSUM") as psM,
                  tc.tile_pool(name="psN", bufs=1, space="PSUM") as psN):
                for g in range(2):
                    if g == 1:
                        # nb1t at the g-seam: fills the PE gap, DVE adds land
                        # after g0 copies, well before ph4 needs them
                        for jt in range(KT):
                            pb = psN.tile([128, T], F32, tag="nb1")
                            nc.tensor.matmul(
                                pb[:], db1_sb[:, jt * 128:(jt + 1) * 128],
                                coefsT_bf[:], start=True, stop=True)
                            nc.vector.tensor_scalar_add(
                                nb1t[:, jt, :], pb[:], b1t_sb[:, jt:jt + 1])
                    for ph in range(4):
                        for kh in (0, 2, 1, 3, 4, 5):
                            iblk = kh * 4 + ph
                            pm = psM.tile([128, 2, 512], F32, tag="m")
                            for jh in range(2):
                                nc.tensor.matmul(
                                    pm[:, jh, 0:384], cb2[g][:],
                                    dw1_sb[:, iblk, :, jh * 384:(jh + 1) * 384],
                                    start=True, stop=True, perf_mode=DR)
                            dst = mxg[g][:, ph, kh, :].rearrange(
                                "p (jh j) -> p jh j", jh=2)
                            if kh < 2:
                                # fold W1 in: (pm/16) + W1/SW  (ktd0)
                                nc.vector.scalar_tensor_tensor(
                                    dst, pm[:, :, 0:384], 1.0 / 16.0,
                                    w1i_sb[:, ph, kh, :].rearrange(
                                        "p (jh j) -> p jh j", jh=2),
                                    op0=MULT, op1=ADD)
                            else:
                                nc.scalar.mul(dst, pm[:, :, 0:384], 1.0 / 16.0)
                        # de-interleave gathers for this (g, ph) column
                        for bg in range(4):
                            deq = nc.sync
                            deq.dma_start(
                                mxcb[g][ph * 32:(ph + 1) * 32, bg, :, :, :],
                                mxg[g][bg * 32:(bg + 1) * 32, ph, :, :])

            for t in range(T):
                nc.vector.tensor_scalar_mul(
                    cdiag[:, t, :], i8bf_sb[:], coefsB[:, t:t + 1])
            mxgp.release()

            # ---------- phase 4: final per-sample pass ----------
            if True:
                with tc.tile_pool(name="psF", bufs=6, space="PSUM") as psF:
                    for b in range(BC):
                        g, bg = b // 4, b % 4
                        for jt in range(KT):
                            pf = psF.tile([128, NPAT], F32, tag="f")
                            for ktd in range(KTD):
                                nc.tensor.matmul(
                                    pf[:],
                                    mxcb[g][:, bg, ktd, :,
                                            jt * 128:(jt + 1) * 128],
                                    xt_sb[:, ktd, :, b * NPAT:(b + 1) * NPAT],
                                    start=(ktd == 0), stop=False,
                                    perf_mode=DR)
                            nc.tensor.matmul(
                                pf[:],
                                w1_sb[:, 1, :, jt * 128:(jt + 1) * 128],
                                xt_sb[:, 1, :, b * NPAT:(b + 1) * NPAT],
                                start=False, stop=False, perf_mode=DR)
                            nc.tensor.matmul(
                                pf[:],
                                w1_sb[:, 2, :, jt * 128:(jt + 1) * 128],
                                xt_sb[:, 2, :, b * NPAT:(b + 1) * NPAT],
                                start=False, stop=True, perf_mode=DR)
                            ro = scr.tile([128, NPAT], F8, tag="ro")
                            if (b * KT + jt) % 8 < 3:   # 18 ACT / 30 DVE
                                nc.scalar.activation(
                                    ro[:], pf[:], RELU,
                                    bias=nb1t[:, jt, b:b + 1],
                                    accum_out=pooln[:, jt, b:b + 1])
                            else:
                                nc.vector.scalar_tensor_tensor(
                                    ro[:], pf[:], nb1t[:, jt, b:b + 1],
                                    zeros_sb[:], op0=ADD, op1=MAX,
                                    accum_out=pooln[:, jt, b:b + 1])

            # ---------- phase 5: layer 2 ----------
            pooln_f8 = sm.tile([128, KTD, 2, 32], F8, tag="poolnf8")
            nc.gpsimd.memset(pooln_f8[:], 0.0)
            nc.scalar.mul(
                pooln_f8[:, :, :, 0:BC].rearrange("p k h b -> p (k h) b"),
                pooln[:], SXW / NPAT)
            pooln_bf = sm.tile([128, KT, BC], BF16, tag="poolnbf")
            nc.gpsimd.tensor_scalar_mul(pooln_bf[:], pooln[:], SXW / NPAT)

            vst = sm.tile([BC, T, D], BF16, tag="vst")
            psV = tc.alloc_tile_pool(name="psV", bufs=2, space="PSUM")
            for t in range(T):
                pv = psV.tile([32, 2, 512], F32, tag="v")
                for eh in range(2):
                    for ktd in range(KTD):
                        nc.tensor.matmul(
                            pv[:, eh, 0:384], pooln_f8[:, ktd, :, :],
                            dw2_sb[:, t, ktd, :, eh * 384:(eh + 1) * 384],
                            start=(ktd == 0), stop=(ktd == KTD - 1),
                            perf_mode=DR)
                if t % 2 == 0:
                    nc.vector.tensor_scalar_mul(
                        vst[:, t, :].rearrange("b (eh e) -> b eh e", eh=2),
                        pv[0:BC, :, 0:384], SW)
                else:
                    nc.scalar.mul(
                        vst[:, t, :].rearrange("b (eh e) -> b eh e", eh=2),
                        pv[0:BC, :, 0:384], SW)

            out_sb = sm.tile([BC, D], F32, tag="out")
            po = psV.tile([BC, 2, 512], F32, tag="v")
            for eh in range(2):
                for kt in range(KT):
                    nc.tensor.matmul(
                        po[:, eh, 0:384], pooln_bf[:, kt, :],
                        w2_sb[:, kt, eh * 384:(eh + 1) * 384],
                        start=(kt == 0), stop=False)
                for t in range(T):
                    nc.tensor.matmul(
                        po[:, eh, 0:384], cdiag[:, t, :],
                        vst[:, t, eh * 384:(eh + 1) * 384],
                        start=False, stop=False)
                nc.tensor.matmul(
                    po[:, eh, 0:384], coefsT_bf[:],
                    db2_sb[:, eh * 384:(eh + 1) * 384],
                    start=False, stop=True)
            nc.vector.tensor_tensor(
                out_sb[:].rearrange("b (eh e) -> b eh e", eh=2),
                po[:, :, 0:384], b2r_sb[:].rearrange("b (eh e) -> b eh e", eh=2),
                op=ADD)
            nc.sync.dma_start(out[:], out_sb[:])
            if debug:
                nc.sync.dma_start(din["dbg_poolb"][:], poolb[:])
                nc.sync.dma_start(din["dbg_coefsB"][:], coefsB[:])
                nc.sync.dma_start(din["dbg_mxcb0"][:], mxcb[0][:])
                nc.sync.dma_start(din["dbg_pooln"][:], pooln[:])
                nc.sync.dma_start(din["dbg_vst"][:], vst[:])
            psV.release()

    if split_waits:
        _split_multi_waits(nc)
    return nc


def prep_inputs(x, W1, b1, W2, b2, dW1, db1, dW2, db2, mw1, mb1, mw2, mb2):
    """Host-side layout prep. Returns per-core in_maps."""
    bf = ml_dtypes.bfloat16
    f8 = ml_dtypes.float8_e4m3
    f32 = np.float32
    x = np.asarray(x, f32); W1 = np.asarray(W1, f32); W2 = np.asarray(W2, f32)
    b1 = np.asarray(b1, f32); b2 = np.asarray(b2, f32)
    dW1 = np.asarray(dW1, f32); dW2 = np.asarray(dW2, f32)
    db1 = np.asarray(db1, f32); db2 = np.asarray(db2, f32)
    mw1 = np.asarray(mw1, f32); mb1 = np.asarray(mb1, f32)
    mw2 = np.asarray(mw2, f32); mb2 = np.asarray(mb2, f32)

    # patches^T: [B, D, NPAT]
    pt = x.reshape(B, 3, 14, P, 14, P).transpose(0, 1, 3, 5, 2, 4)
    pt = np.ascontiguousarray(pt).reshape(B, D, NPAT)

    # shared (replicated) tensors
    w1_c = np.ascontiguousarray(
        (W1 / SW).reshape(KTD, 2, 128, D).transpose(2, 0, 1, 3)).astype(f8)
    w2_c = np.ascontiguousarray(
        W2.reshape(KT, 128, D).transpose(1, 0, 2)).astype(bf)
    # dw1[(t,slo), iblk, shi, j] = dW1[t, iblk*32+shi*16+slo, j]/SW
    d = (dW1 / SW).reshape(T, 24, 2, P, D)       # [t, iblk, shi, slo, j]
    dw1_c = np.ascontiguousarray(
        d.transpose(0, 3, 1, 2, 4).reshape(128, 24, 2, D)).astype(f8)
    # w1i[(bg,s'), ph, kh, j] = W1[(kh*4+ph)*32+s', j]/SW  (kh<2: ktd0)
    w1i_c = np.zeros((128, 4, 2, D), np.float32)
    for ph in range(4):
        for kh in range(2):
            blk = (W1 / SW)[(kh * 4 + ph) * 32:(kh * 4 + ph) * 32 + 32, :]
            for bg in range(4):
                w1i_c[bg * 32:(bg + 1) * 32, ph, kh, :] = blk
    w1i_c = w1i_c.astype(f8)
    # dw2[k_local, t, ktd, hi, e] = dW2[t, ktd*256+hi*128+k_local, e]/SW
    dw2_c = np.ascontiguousarray(
        (dW2 / SW).reshape(T, KTD, 2, 128, D).transpose(3, 0, 1, 2, 4)
    ).astype(f8)
    db1_c = (db1 / SXW).astype(bf)
    db2_c = db2.astype(bf)
    b1t_c = np.ascontiguousarray((b1 / SXW).reshape(KT, 128).T).astype(f32)
    b2t_c = np.ascontiguousarray(b2.reshape(KT, 128).T).astype(f32)
    b2t4_c = np.repeat(b2t_c[:, :, None], 4, axis=2)
    b2r_c = np.tile(b2, (BC, 1))
    mw1_c = np.ascontiguousarray(
        mw1.reshape(KT, 128, HM).transpose(1, 0, 2)).astype(bf)
    mb1t_c = np.zeros((128, 2), f32)
    mb1t_c[:, 0] = mb1[:128]
    mb1t_c[:64, 1] = mb1[128:]
    mw2_c = np.zeros((128, 2, T), f32)
    mw2_c[:, 0, :] = mw2[:128]
    mw2_c[:64, 1, :] = mw2[128:]
    mw2_c = mw2_c.astype(bf)
    mb2r_c = np.tile(mb2, (BC, 1)).astype(f32)
    iexp16_c = np.repeat(np.eye(T, dtype=f32) * 16.0, P, axis=1).astype(bf)
    # mask32[(t,slo), shi, s'] = (s' == shi*16+slo)
    m32 = np.zeros((P, 2, 32), f32)
    for slo in range(P):
        for shi in range(2):
            m32[slo, shi, shi * P + slo] = 1.0
    mask32_c = np.tile(m32, (T, 1, 1)).astype(bf)
    i8_c = np.eye(T, dtype=f32)

    shared = dict(
        w1=w1_c, w1i=w1i_c, w2=w2_c, dw1=dw1_c, dw2=dw2_c, db1=db1_c,
        db2=db2_c,
        b1t=b1t_c, b2t=b2t_c, b2t4=b2t4_c, b2r=b2r_c, mw1=mw1_c,
        mb1t=mb1t_c,
        mw2=mw2_c, mb2r=mb2r_c,
        iexp16=iexp16_c, mask32=mask32_c, i8=i8_c, i8bf=i8_c.astype(bf),
    )

    in_maps = []
    for c in range(NCORES):
        ptc = pt[c * BC:(c + 1) * BC]                  # [BC, D, NPAT]
        # xt[p, ktd, hi, (b,n)] = ptc[b, ktd*256+hi*128+p, n]/SX
        xt_c = np.ascontiguousarray(
            (ptc / SX).reshape(BC, KTD, 2, 128, NPAT).transpose(3, 1, 2, 0, 4)
        ).reshape(128, KTD, 2, NB).astype(f8)
        m = dict(shared)
        m["xt"] = xt_c
        in_maps.append(m)
    return in_maps


_NC_CACHE = {}


def kernel(**inputs) -> np.ndarray:
    _apply_tile_patch()
    if "nc" not in _NC_CACHE:
        _NC_CACHE["nc"] = build_kernel()
    nc = _NC_CACHE["nc"]
    in_maps = prep_inputs(**inputs)
    res = run_bass_kernel_spmd(nc, in_maps, core_ids=list(range(NCORES)))
    return np.concatenate([r["out"] for r in res.results], axis=0)
